# revision 29
# baseline (speedup 1.0000x reference)
"""Trainium2 Bass kernel for a ViT attention block (LN -> QKV -> RoPE -> attn -> out-proj).

Sharding: 8 cores = 2 batches x 4 head-groups (4 heads each). Each core computes
a partial out-projection (its 4 heads) for one batch, transposed as [D, N] bf16.
Host sums the 4 partials per batch and transposes back. LayerNorm gamma/beta are
folded into the QKV weights/bias on the host.

Deep-pipelined schedule (v2):
  - Per token group g (4+4+4+4+1 of the 17 token tiles): LN tiles (stats on DVE,
    normalize on ACT with per-partition scale/bias), PE-transpose into xT, then
    immediately V-projection for the group's k-tiles and Q/K projection+RoPE for
    heads 0-1 (f=0,2). So attention for head pair (0,1) starts as soon as group 4
    lands, ~3x earlier than a phase-sequential schedule.
  - Attention per jj-block per head pair: the two heads' K=64 dots matmuls sit in
    disjoint PE row groups (partitions 0:64 / 64:128, tile_position auto-derived)
    and are emitted back-to-back so they run concurrently. exp on ACT is the
    critical engine; attn@v (M=65 with appended ones column giving the softmax
    denominator) follows per head.
  - Q/K projection+RoPE for heads 2-3 (f=1,3) and block-0's out-projection are
    side-injected between k-tiles of the ACT-bound attention loops (PE/DVE slack).
  - Denominators: merged per pair into one tile, bf16->f32 cast, single
    reciprocal_approx_fast, cast back; normalize via ones outer-product broadcast
    (PE) + DVE multiply; out-proj accumulates the 256 head dims; out as [D,N] bf16.
PSUM: "dots" ring 2x[128,1024]f32 (4 banks; also transposes, side psq, pp, psb),
"pso" ring 4x[65,512]f32 (4 banks) for the pair's attn@v accumulators.
All matmuls bf16 with f32 PSUM accumulation.
"""

import sys

sys.path.insert(0, "/opt/trn_rl_repo")

import numpy as np
import ml_dtypes

import concourse.bacc as bacc
import concourse.mybir as mybir
import concourse.tile as tile
from concourse.bass_utils import run_bass_kernel_spmd

F32 = mybir.dt.float32
BF16 = mybir.dt.bfloat16
AF = mybir.ActivationFunctionType
OP = mybir.AluOpType
BF = ml_dtypes.bfloat16

B, N, D = 2, 2049, 1024
DH = 64
HPC = 4  # heads per core
NT = 17  # 128-token tiles (padded to 2176)
TPAD = NT * 128
SCALE = DH ** -0.5
# q-column blocks [offset, width]; the tail block is the single real token 2048
JJ = [(0, 1024), (1024, 1024), (2048, 1)]
J5 = [(0, 512), (512, 512), (1024, 512), (1536, 512), (2048, 128)]
# token groups backing the 5 xT tiles (4+4+4+4+1 of the 17 token tiles)
TG = [(0, 512), (512, 512), (1024, 512), (1536, 512), (2048, 128)]


def _subs(jw):
    return [(s, min(512, jw - s)) for s in range(0, jw, 512)]


def _tg_of(col):
    return min(col // 512, 4)


def _build():
    nc = bacc.Bacc("TRN2", target_bir_lowering=False, debug=False, num_devices=8)

    x_d = nc.declare_dram_parameter("x", [N, D], F32, False)
    wqk_d = nc.declare_dram_parameter("wqk", [D, 512], BF16, False)
    wv_d = nc.declare_dram_parameter("wv", [D, 256], BF16, False)
    wo_d = nc.declare_dram_parameter("wo", [256, D], BF16, False)
    bqk_d = nc.declare_dram_parameter("bqk", [1, 512], BF16, False)
    bv_d = nc.declare_dram_parameter("bv", [1, 256], BF16, False)
    cos_d = nc.declare_dram_parameter("cos2", [128, TPAD], BF16, False)
    sin_d = nc.declare_dram_parameter("sinf2", [128, TPAD], BF16, False)
    idn_d = nc.declare_dram_parameter("ident", [128, 128], BF16, False)
    out_d = nc.declare_dram_parameter("out", [D, N], BF16, True)

    with tile.TileContext(nc) as tc:
        with (
            tc.tile_pool(name="const", bufs=1) as cpool,
            tc.tile_pool(name="persist", bufs=1) as ppool,
            tc.tile_pool(name="work", bufs=2) as wpool,
            tc.tile_pool(name="psum", bufs=2, space="PSUM") as pspool,
        ):
            # ---------------- constants ----------------
            wqk_sb = [cpool.tile([128, 512], BF16, tag=f"wqk{c}", name=f"wqk{c}") for c in range(8)]
            wv_sb = [cpool.tile([128, 256], BF16, tag=f"wv{c}", name=f"wv{c}") for c in range(8)]
            wo_sb = [cpool.tile([128, 1024], BF16, tag=f"wo{c}", name=f"wo{c}") for c in range(2)]
            bqk_sb = cpool.tile([1, 512], BF16, tag="bqk", name="bqk")
            bv_sb = cpool.tile([1, 256], BF16, tag="bv", name="bv")
            cos_sb = cpool.tile([128, TPAD], BF16, tag="cos", name="cos")
            sin_sb = cpool.tile([128, TPAD], BF16, tag="sin", name="sin")
            idn_sb = cpool.tile([128, 128], BF16, tag="idn", name="idn")
            ones_sb = cpool.tile([1, TPAD], BF16, tag="ones", name="ones")

            nc.sync.dma_start(out=idn_sb[:], in_=idn_d[:])

            def _load_weights():
                for c in range(8):
                    nc.sync.dma_start(out=wqk_sb[c][:], in_=wqk_d[c * 128:(c + 1) * 128, :])
                    nc.sync.dma_start(out=wv_sb[c][:], in_=wv_d[c * 128:(c + 1) * 128, :])

            def _load_consts():
                for c in range(2):
                    nc.gpsimd.dma_start(out=wo_sb[c][:], in_=wo_d[c * 128:(c + 1) * 128, :])
                nc.gpsimd.dma_start(out=bqk_sb[:], in_=bqk_d[:])
                nc.gpsimd.dma_start(out=bv_sb[:], in_=bv_d[:])
                nc.gpsimd.dma_start(out=cos_sb[:], in_=cos_d[:])
                nc.gpsimd.dma_start(out=sin_sb[:], in_=sin_d[:])

            nc.vector.memset(ones_sb[:], 1.0)
            eps_sb = cpool.tile([128, 1], F32, tag="eps", name="eps")
            nc.vector.memset(eps_sb[:], 1e-5)
            ones64_sb = cpool.tile([128, 64], BF16, tag="ones64", name="ones64")
            nc.vector.memset(ones64_sb[:], 1.0)

            # ---------------- persistent activations ----------------
            xTg = [
                ppool.tile([128, 8 * tw], BF16, tag=f"xT{g}", name=f"xT{g}")
                for g, (to, tw) in enumerate(TG)
            ]
            xT3 = [
                xTg[g][:, :].rearrange("p (c t) -> p c t", c=8) for g in range(5)
            ]

            def xslice(c, jo, jw):
                g = _tg_of(jo)
                to, tw = TG[g]
                assert jo + jw <= to + tw
                return xT3[g][:, c, jo - to:jo - to + jw]

            # qkT tiles: 0,1 = q head-pairs (h01, h23); 2,3 = k head-pairs
            qkT_sb = [ppool.tile([128, TPAD], BF16, tag=f"qkT{f}", name=f"qkT{f}") for f in range(4)]
            vaug_sb = [ppool.tile([128, 260], BF16, tag=f"v{k}", name=f"v{k}") for k in range(NT)]

            # ---------------- phase A: LayerNorm + transpose (per tile) ----------
            def emit_ln_tile(i):
                xa = wpool.tile([128, D], F32, tag="xa", name="xa", bufs=3)
                if i < 16:
                    eng = nc.sync if i % 2 == 0 else nc.gpsimd
                    eng.dma_start(out=xa[:], in_=x_d[i * 128:(i + 1) * 128, :])
                else:
                    nc.vector.memset(xa[:], 0.0)
                    nc.sync.dma_start(out=xa[0:1, :], in_=x_d[2048:2049, :])
                if i == 0:
                    _load_weights()
                if i == 1:
                    _load_consts()
                stats = wpool.tile([128, 12], F32, tag="stats", name="stats", bufs=3)
                mv = wpool.tile([128, 2], F32, tag="mv", name="mv", bufs=4)
                nc.vector.bn_stats(stats[:, 0:6], xa[:, 0:512])
                nc.vector.bn_stats(stats[:, 6:12], xa[:, 512:1024])
                nc.vector.bn_aggr(mv[:], stats[:])
                std = wpool.tile([128, 1], F32, tag="std", name="std", bufs=4)
                rstd = wpool.tile([128, 1], F32, tag="rstd", name="rstd", bufs=4)
                nmurstd = wpool.tile([128, 1], F32, tag="murstd", name="nmurstd")
                nc.scalar.activation(std[:], mv[:, 1:2], AF.Sqrt, bias=eps_sb[:])
                nc.vector.reciprocal(rstd[:], std[:])
                nc.vector.scalar_tensor_tensor(
                    nmurstd[:], mv[:, 0:1], -1.0, rstd[:], OP.mult, OP.mult
                )
                # xn = rstd*x - mu*rstd on the scalar engine (idle in this phase)
                xn = wpool.tile([128, D], BF16, tag="xn", name="xn", bufs=4)
                nc.scalar.activation(
                    xn[:], xa[:], AF.Identity, bias=nmurstd[:], scale=rstd[:]
                )
                g = _tg_of(i * 128)
                to, tw = TG[g]
                for s in range(2):
                    pst = pspool.tile([128, 512], BF16, tag="dots", name="pst", bufs=2)
                    for c in range(4):
                        nc.tensor.transpose(
                            pst[:, c * 128:(c + 1) * 128],
                            xn[:, (4 * s + c) * 128:(4 * s + c + 1) * 128],
                            idn_sb[:],
                        )
                    nc.vector.tensor_copy(
                        xT3[g][:, 4 * s:4 * s + 4, i * 128 - to:(i + 1) * 128 - to],
                        pst[:, :].rearrange("p (c t) -> p c t", c=4),
                    )

            # ---------------- V projection for one k-tile ----------------
            def emit_v_tile(k):
                # pso ring is idle during W1 — use it for V-projection PSUM
                psv = pspool.tile([128, 256], F32, tag="pso", name="psv", bufs=4)
                for c in range(8):
                    nc.tensor.matmul(
                        psv[:],
                        xslice(c, k * 128, 128),
                        wv_sb[c][:],
                        start=(c == 0),
                        stop=False,
                    )
                nc.tensor.matmul(
                    psv[:],
                    ones_sb[:, k * 128:(k + 1) * 128],
                    bv_sb[:],
                    start=False,
                    stop=True,
                )
                va = vaug_sb[k]
                va3 = va[:, :].rearrange("p (a b) -> p a b", a=4)
                if k < 16:
                    nc.scalar.copy(
                        va3[:, :, 0:64], psv[:, :].rearrange("p (a b) -> p a b", a=4)
                    )
                    nc.vector.memset(va3[:, :, 64:65], 1.0)
                else:
                    # only token 2048 is real; zero rows kill padded keys
                    nc.vector.memset(va[:], 0.0)
                    va3r = va[0:1, :].rearrange("p (a b) -> p a b", a=4)
                    nc.scalar.copy(
                        va3r[:, :, 0:64],
                        psv[0:1, :].rearrange("p (a b) -> p a b", a=4),
                    )
                    nc.vector.memset(va3r[:, :, 64:65], 1.0)

            # ------------- Q/K projection + RoPE for one J5 chunk -------------
            def emit_qk_chunk(f, jo, jw, on_act):
                # W1 chunks (on_act) ride the idle pso ring; attention-injected
                # chunks must use the dots ring (pso ring is held by attn@v)
                psq = (pspool.tile([128, 512], F32, tag="pso", name="psq", bufs=4)
                       if on_act else
                       pspool.tile([128, 512], F32, tag="dots", name="psq", bufs=2))
                for c in range(8):
                    nc.tensor.matmul(
                        psq[:, 0:jw],
                        wqk_sb[c][:, f * 128:(f + 1) * 128],
                        xslice(c, jo, jw),
                        start=(c == 0),
                        stop=False,
                    )
                nc.tensor.matmul(
                    psq[:, 0:jw],
                    bqk_sb[:, f * 128:(f + 1) * 128],
                    ones_sb[:, jo:jo + jw],
                    start=False,
                    stop=True,
                )
                qf = wpool.tile([128, 512], BF16, tag="qf", name="qf", bufs=3)
                if on_act:
                    nc.scalar.copy(qf[:, 0:jw], psq[:, 0:jw])
                else:
                    nc.vector.tensor_copy(qf[:, 0:jw], psq[:, 0:jw])
                # rotate-half partner via partition-shifting DMAs (spread queues)
                qs = wpool.tile([128, 512], BF16, tag="qs", name="qs", bufs=3)
                e0, e1 = (nc.sync, nc.gpsimd) if on_act else (nc.sync, nc.gpsimd)
                e0.dma_start(out=qs[0:32, 0:jw], in_=qf[32:64, 0:jw])
                e1.dma_start(out=qs[32:64, 0:jw], in_=qf[0:32, 0:jw])
                e0.dma_start(out=qs[64:96, 0:jw], in_=qf[96:128, 0:jw])
                e1.dma_start(out=qs[96:128, 0:jw], in_=qf[64:96, 0:jw])
                t1 = wpool.tile([128, 512], BF16, tag="t1", name="t1", bufs=2)
                t2 = wpool.tile([128, 512], BF16, tag="t2", name="t2", bufs=2)
                nc.vector.tensor_mul(
                    t1[:, 0:jw], qf[:, 0:jw], cos_sb[:, jo:jo + jw]
                )
                nc.vector.tensor_mul(
                    t2[:, 0:jw], qs[:, 0:jw], sin_sb[:, jo:jo + jw]
                )
                nc.vector.tensor_add(
                    qkT_sb[f][:, jo:jo + jw], t1[:, 0:jw], t2[:, 0:jw]
                )

            # ---------------- fused phase A+B pipeline ----------------
            for g in range(5):
                to, tw = TG[g]
                for i in range(to // 128, (to + tw) // 128):
                    emit_ln_tile(i)
                for k in range(to // 128, (to + tw) // 128):
                    emit_v_tile(k)
                emit_qk_chunk(0, to, tw, on_act=True)
                emit_qk_chunk(2, to, tw, on_act=True)

            # ---------------- attention ----------------
            def att_pair(jo, jw, p, side, po_every=2, po_rem=1):
                """Heads 2p, 2p+1: row-group-packed dots + exp + attn@v.
                Returns (oev per head, dnp merged-denominator tile)."""
                qt, kt = qkT_sb[p], qkT_sb[2 + p]
                pso = [
                    [
                        pspool.tile([65, 512], F32, tag="pso", name=f"pso{hh}{si}", bufs=4)
                        for si in range(len(_subs(jw)))
                    ]
                    for hh in range(2)
                ]
                dnp = wpool.tile([64, 1024], BF16, tag="dnp", name="dnp", bufs=2)

                def emit_attnv(k, exs):
                    for hh in range(2):
                        h = 2 * p + hh
                        for si, (so, sw) in enumerate(_subs(jw)):
                            nc.tensor.matmul(
                                pso[hh][si][:, 0:sw],
                                vaug_sb[k][:, h * 65:h * 65 + 65],
                                exs[hh][:, so:so + sw],
                                start=(k == 0),
                                stop=(k == NT - 1),
                                skip_group_check=True,
                            )

                # attn@v runs one k-tile behind dots/exp so the in-order PE
                # queue never stalls waiting on the current k-tile's exp
                prev = None
                for k in range(NT):
                    psds = [
                        pspool.tile([128, 1024], F32, tag="dots", name=f"psd{hh}", bufs=2)
                        for hh in range(2)
                    ]
                    # interleave the two heads' subs so their disjoint row
                    # groups overlap on the PE array
                    for so, sw in _subs(jw):
                        for hh in range(2):
                            po = hh * 64
                            nc.tensor.matmul(
                                psds[hh][:, so:so + sw],
                                kt[po:po + 64, k * 128:(k + 1) * 128],
                                qt[po:po + 64, jo + so:jo + so + sw],
                            )
                    exs = []
                    for hh in range(2):
                        ex = wpool.tile([128, 1024], BF16, tag="ex", name="ex", bufs=8)
                        nc.scalar.activation(
                            ex[:, 0:jw], psds[hh][:, 0:jw], AF.Exp, scale=SCALE
                        )
                        exs.append(ex)
                    if prev is not None:
                        emit_attnv(prev[0], prev[1])
                    if side and k % po_every == po_rem:
                        side.pop(0)()
                    prev = (k, exs)
                emit_attnv(prev[0], prev[1])
                oev_l = []
                for hh in range(2):
                    h = 2 * p + hh
                    oev = wpool.tile([65, 1024], BF16, tag="oev", name="oev", bufs=8)
                    for si, (so, sw) in enumerate(_subs(jw)):
                        nc.vector.tensor_copy(oev[:, so:so + sw], pso[hh][si][:, 0:sw])
                    oev_l.append(oev)
                    nc.gpsimd.dma_start(
                        out=dnp[32 * hh:32 * hh + 1, 0:jw],
                        in_=oev[64:65, 0:jw],
                    )
                return oev_l, dnp

            def att_head1(jo, h, pso_l, dnp_l):
                """Single-column (jw=1) attention for head h."""
                qt = qkT_sb[h // 2]
                ktile = qkT_sb[2 + h // 2]
                po = (h % 2) * 64
                pso = pspool.tile([65, 512], F32, tag="pso", name="pso1", bufs=4)
                psd = pspool.tile([128, NT], F32, tag="dots", name="psdt", bufs=2)
                for k in range(NT):
                    nc.tensor.matmul(
                        psd[:, k:k + 1],
                        ktile[po:po + 64, k * 128:(k + 1) * 128],
                        qt[po:po + 64, jo:jo + 1],
                    )
                ex = wpool.tile([128, 1024], BF16, tag="ex", name="ex", bufs=8)
                nc.scalar.activation(ex[:, 0:NT], psd[:], AF.Exp, scale=SCALE)
                for k in range(NT):
                    nc.tensor.matmul(
                        pso[:, 0:1],
                        vaug_sb[k][:, h * 65:h * 65 + 65],
                        ex[:, k:k + 1],
                        start=(k == 0),
                        stop=(k == NT - 1),
                        skip_group_check=True,
                    )
                oev = wpool.tile([65, 64], BF16, tag="oevt", name="oevt", bufs=4)
                nc.vector.tensor_copy(oev[:, 0:1], pso[:, 0:1])
                pso_l.append(oev)
                dnp = dnp_l[h // 2]
                nc.gpsimd.dma_start(
                    out=dnp[32 * (h % 2):32 * (h % 2) + 1, 0:1],
                    in_=oev[64:65, 0:1],
                )

            def epi_recip(jw, dnp):
                # one full-width bf16->f32 cast, approx reciprocal, cast back
                dnb = wpool.tile([64, 1024], BF16, tag="dnb", name="dnb", bufs=6)
                dnf = wpool.tile([64, 1024], F32, tag="dnf", name="dnf", bufs=1)
                dnr = wpool.tile([64, 1024], F32, tag="dnr", name="dnr", bufs=1)
                nc.vector.tensor_copy(dnf[:, 0:jw], dnp[:, 0:jw])
                nc.vector.reciprocal_approx_fast(out=dnr[:, 0:jw], in_=dnf[:, 0:jw])
                nc.vector.tensor_copy(dnb[:, 0:jw], dnr[:, 0:jw])
                return dnb

            def norm_chunk(h, so, sw, oev, dnb, an):
                hr = 32 * (h % 2)
                psb = pspool.tile([64, 512], F32, tag="dots", name="psb", bufs=2)
                nc.tensor.matmul(
                    psb[:, 0:sw],
                    ones64_sb[hr:hr + 1, :],
                    dnb[hr:hr + 1, so:so + sw],
                )
                if h % 2 == 0:
                    nc.vector.tensor_mul(
                        an[h // 2][0:64, so:so + sw],
                        oev[0:64, so:so + sw],
                        psb[:, 0:sw],
                    )
                else:
                    nt = wpool.tile([64, 512], BF16, tag="nt", name="nt", bufs=4)
                    nc.vector.tensor_mul(
                        nt[:, 0:sw], oev[0:64, so:so + sw], psb[:, 0:sw]
                    )
                    nc.gpsimd.dma_start(
                        out=an[h // 2][64:128, so:so + sw], in_=nt[:, 0:sw]
                    )

            def epi_norm(jw, p, oev_l, dnb, an):
                for hh in range(2):
                    for so, sw in _subs(jw):
                        norm_chunk(2 * p + hh, so, sw, oev_l[hh], dnb, an)

            def emit_op_chunk(jo, jw, an, of, so, sw, final=False):
                swr = min(sw, max(0, N - (jo + so)))
                pp = pspool.tile([128, 512], F32, tag="dots", name="pp", bufs=2)
                nc.tensor.matmul(
                    pp[:, 0:sw],
                    wo_sb[0][:, of * 128:(of + 1) * 128],
                    an[0][:, so:so + sw],
                    start=True,
                    stop=False,
                )
                nc.tensor.matmul(
                    pp[:, 0:sw],
                    wo_sb[1][:, of * 128:(of + 1) * 128],
                    an[1][:, so:so + sw],
                    start=False,
                    stop=True,
                )
                oo = wpool.tile([128, 512], BF16, tag="oo", name="oo", bufs=3)
                if final and of % 2 == 1:
                    nc.scalar.copy(oo[:, 0:swr], pp[:, 0:swr])
                else:
                    nc.vector.tensor_copy(oo[:, 0:swr], pp[:, 0:swr])
                nc.sync.dma_start(
                    out=out_d[of * 128:(of + 1) * 128, jo + so:jo + so + swr],
                    in_=oo[:, 0:swr],
                )

            # side work: Q/K projection + RoPE for heads 2-3, in half-chunks so
            # each injection fits the PE slack of one ACT-bound k-tile slot.
            # Order: K23 (pair23's dots need all of kT23), then Q23 g4 (the
            # cls-token block needs it), then Q23 g0-g3.
            side = []
            for f, g in ([(3, g) for g in range(5)] + [(1, 4)] +
                         [(1, g) for g in range(4)]):
                side.append(
                    lambda f=f, g=g: emit_qk_chunk(f, TG[g][0], TG[g][1], on_act=False)
                )

            def mk_an(nm):
                return [
                    wpool.tile([128, 1024], BF16, tag=f"an{nm}{ch}", name=f"an{nm}{ch}", bufs=1)
                    for ch in range(2)
                ]

            an_b0, an_b1, an_1 = mk_an("b0"), mk_an("b1"), mk_an("t")

            # ---- block b0 (q cols 0-1023): psq23 chunks side-injected ----
            oev01_b0, dnp0_b0 = att_pair(0, 1024, 0, side)
            oev23_b0, dnp1_b0 = att_pair(0, 1024, 1, side)
            # denominator reciprocals early (DVE-only, no PE queue blockage)
            dnb0_b0 = epi_recip(1024, dnp0_b0)
            dnb1_b0 = epi_recip(1024, dnp1_b0)

            # ---- jw=1 block (cls-token q column 2048) at the b0/b1 boundary ----
            dnp_l = [
                wpool.tile([64, 1024], BF16, tag="dnp", name=f"dnp1{p}", bufs=2)
                for p in range(2)
            ]
            oev1 = []
            for h in range(4):
                att_head1(2048, h, oev1, dnp_l)
            dnb_1 = [epi_recip(1, dnp_l[p]) for p in range(2)]

            def _norm_jw1():
                # deferred into b1-p01's side pops so the PE queue is not
                # blocked at the block boundary waiting on the DVE recip chain
                for p in range(2):
                    for hh in range(2):
                        norm_chunk(2 * p + hh, 0, 1, oev1[2 * p + hh], dnb_1[p], an_1)

            side += [_norm_jw1] + [
                (lambda of=of: emit_op_chunk(2048, 1, an_1, of, 0, 1))
                for of in range(8)
            ]

            # ---- block b1 (q cols 1024-2047): leftover side + jw1 outproj in
            #      p01, b0's epilogue between the pairs, b0's outproj in p23 ----
            oev01_b1, dnp0_b1 = att_pair(1024, 1024, 0, side, po_every=1, po_rem=0)
            epi_norm(1024, 0, oev01_b0, dnb0_b0, an_b0)
            epi_norm(1024, 1, oev23_b0, dnb1_b0, an_b0)
            side += [
                (lambda of=of, so=so, sw=sw:
                 emit_op_chunk(0, 1024, an_b0, of, so, sw))
                for of in range(8)
                for so, sw in _subs(1024)
            ]
            oev23_b1, dnp1_b1 = att_pair(1024, 1024, 1, side, po_every=1, po_rem=0)
            dnb0_b1 = epi_recip(1024, dnp0_b1)
            dnb1_b1 = epi_recip(1024, dnp1_b1)
            while side:
                side.pop(0)()
            # tail: normalize and out-project by column half so the second
            # half's norm (DVE) overlaps the first half's outproj (PE)
            for so, sw in _subs(1024):
                for hh in range(2):
                    norm_chunk(0 + hh, so, sw, oev01_b1[hh], dnb0_b1, an_b1)
                    norm_chunk(2 + hh, so, sw, oev23_b1[hh], dnb1_b1, an_b1)
                for of in range(8):
                    emit_op_chunk(1024, 1024, an_b1, of, so, sw,
                                  final=(of % 2 == 1))

    nc.compile()
    return nc


_NC = None


def _get_nc():
    global _NC
    if _NC is None:
        _NC = _build()
    return _NC


def _make_inputs(x, ln_gamma, ln_beta, w_qkv, w_out):
    w_eff = (w_qkv * ln_gamma[:, None].astype(np.float32)).astype(np.float32)
    b_all = (ln_beta.astype(np.float32) @ w_qkv.astype(np.float32)).astype(np.float32)

    inv = 1.0 / (10000.0 ** (np.arange(0, 64, 2, dtype=np.float64) / 64.0))
    fr = np.arange(2048, dtype=np.float64)[:, None] * inv[None, :]
    cos64 = np.concatenate([np.cos(fr), np.cos(fr)], axis=1).T  # [64, 2048]
    sinf64 = np.concatenate([-np.sin(fr), np.sin(fr)], axis=1).T
    cos2 = np.ones((128, TPAD), np.float32)
    sinf2 = np.zeros((128, TPAD), np.float32)
    cos2[:, 1:2049] = np.tile(cos64, (2, 1)).astype(np.float32)
    sinf2[:, 1:2049] = np.tile(sinf64, (2, 1)).astype(np.float32)
    ident = np.eye(128, dtype=BF)

    in_maps = []
    for c in range(8):
        b, g = c // 4, c % 4
        cols = slice(256 * g, 256 * g + 256)
        wqk = np.concatenate(
            [w_eff[:, 0:1024][:, cols], w_eff[:, 1024:2048][:, cols]], axis=1
        ).astype(BF)
        wv = w_eff[:, 2048:3072][:, cols].astype(BF)
        wo = w_out[cols, :].astype(BF)
        bqk = np.concatenate([b_all[0:1024][cols], b_all[1024:2048][cols]])[None, :].astype(BF)
        bv = b_all[2048:3072][cols][None, :].astype(BF)
        in_maps.append(
            {
                "x": np.ascontiguousarray(x[b]).astype(np.float32),
                "wqk": np.ascontiguousarray(wqk),
                "wv": np.ascontiguousarray(wv),
                "wo": np.ascontiguousarray(wo),
                "bqk": bqk,
                "bv": bv,
                "cos2": cos2.astype(BF),
                "sinf2": sinf2.astype(BF),
                "ident": ident,
            }
        )
    return in_maps


def kernel(x, ln_gamma, ln_beta, w_qkv, w_out, _trace=False, _trace_kwargs=None):
    nc = _get_nc()
    in_maps = _make_inputs(x, ln_gamma, ln_beta, w_qkv, w_out)
    res = run_bass_kernel_spmd(
        nc, in_maps, core_ids=list(range(8)), trace=_trace,
        **(_trace_kwargs or {}),
    )
    out = np.zeros((B, N, D), np.float32)
    for c in range(8):
        out[c // 4] += np.asarray(res.results[c]["out"], np.float32).T
    if _trace:
        return out, res
    return out


# revision 31
# speedup vs baseline: 1.1363x; 1.1363x over previous
"""Trainium2 Bass kernel for a ViT attention block (LN -> QKV -> RoPE -> attn -> out-proj).

Sharding: 8 cores = 2 batches x 4 head-groups (4 heads each). Each core computes
a partial out-projection (its 4 heads) for one batch, transposed as [D, N] bf16.
Host sums the 4 partials per batch and transposes back. LayerNorm gamma/beta are
folded into the QKV weights/bias on the host.

Deep-pipelined schedule (v2):
  - Per token group g (4+4+4+4+1 of the 17 token tiles): LN tiles (stats on DVE,
    normalize on ACT with per-partition scale/bias), PE-transpose into xT, then
    immediately V-projection for the group's k-tiles and Q/K projection+RoPE for
    heads 0-1 (f=0,2). So attention for head pair (0,1) starts as soon as group 4
    lands, ~3x earlier than a phase-sequential schedule.
  - Attention per jj-block per head pair: the two heads' K=64 dots matmuls sit in
    disjoint PE row groups (partitions 0:64 / 64:128, tile_position auto-derived)
    and are emitted back-to-back so they run concurrently. exp on ACT is the
    critical engine; attn@v (M=65 with appended ones column giving the softmax
    denominator) follows per head.
  - Q/K projection+RoPE for heads 2-3 (f=1,3) and block-0's out-projection are
    side-injected between k-tiles of the ACT-bound attention loops (PE/DVE slack).
  - Denominators: merged per pair into one tile, bf16->f32 cast, single
    reciprocal_approx_fast, cast back; normalize via ones outer-product broadcast
    (PE) + DVE multiply; out-proj accumulates the 256 head dims; out as [D,N] bf16.
PSUM: "dots" ring 2x[128,1024]f32 (4 banks; also transposes, side psq, pp, psb),
"pso" ring 4x[65,512]f32 (4 banks) for the pair's attn@v accumulators.
All matmuls bf16 with f32 PSUM accumulation.
"""

import sys

sys.path.insert(0, "/opt/trn_rl_repo")

import numpy as np
import ml_dtypes

import concourse.bacc as bacc
import concourse.mybir as mybir
import concourse.tile as tile
from concourse.bass_utils import run_bass_kernel_spmd

F32 = mybir.dt.float32
BF16 = mybir.dt.bfloat16
AF = mybir.ActivationFunctionType
OP = mybir.AluOpType
BF = ml_dtypes.bfloat16

B, N, D = 2, 2049, 1024
DH = 64
HPC = 4  # heads per core
NT = 17  # 128-token tiles (padded to 2176)
TPAD = NT * 128
SCALE = DH ** -0.5
# q-column blocks [offset, width]; the tail block is the single real token 2048
JJ = [(0, 1024), (1024, 1024), (2048, 1)]
J5 = [(0, 512), (512, 512), (1024, 512), (1536, 512), (2048, 128)]
# token groups backing the 5 xT tiles (4+4+4+4+1 of the 17 token tiles)
TG = [(0, 512), (512, 512), (1024, 512), (1536, 512), (2048, 128)]


def _subs(jw):
    return [(s, min(512, jw - s)) for s in range(0, jw, 512)]


def _tg_of(col):
    return min(col // 512, 4)


def _build():
    nc = bacc.Bacc("TRN2", target_bir_lowering=False, debug=False, num_devices=8)

    x_d = nc.declare_dram_parameter("x", [N, D], F32, False)
    wqk_d = nc.declare_dram_parameter("wqk", [D, 512], BF16, False)
    wv_d = nc.declare_dram_parameter("wv", [D, 256], BF16, False)
    wo_d = nc.declare_dram_parameter("wo", [256, D], BF16, False)
    bqk_d = nc.declare_dram_parameter("bqk", [1, 512], BF16, False)
    bv_d = nc.declare_dram_parameter("bv", [1, 256], BF16, False)
    cos_d = nc.declare_dram_parameter("cos2", [128, TPAD], BF16, False)
    sin_d = nc.declare_dram_parameter("sinf2", [128, TPAD], BF16, False)
    idn_d = nc.declare_dram_parameter("ident", [128, 128], BF16, False)
    out_d = nc.declare_dram_parameter("out", [D, N], BF16, True)

    with tile.TileContext(nc) as tc:
        with (
            tc.tile_pool(name="const", bufs=1) as cpool,
            tc.tile_pool(name="persist", bufs=1) as ppool,
            tc.tile_pool(name="work", bufs=2) as wpool,
            tc.tile_pool(name="psum", bufs=2, space="PSUM") as pspool,
        ):
            # ---------------- constants ----------------
            wqk_sb = [cpool.tile([128, 512], BF16, tag=f"wqk{c}", name=f"wqk{c}") for c in range(8)]
            wv_sb = [cpool.tile([128, 256], BF16, tag=f"wv{c}", name=f"wv{c}") for c in range(8)]
            wo_sb = [cpool.tile([128, 1024], BF16, tag=f"wo{c}", name=f"wo{c}") for c in range(2)]
            bqk_sb = cpool.tile([1, 512], BF16, tag="bqk", name="bqk")
            bv_sb = cpool.tile([1, 256], BF16, tag="bv", name="bv")
            cos_sb = cpool.tile([128, TPAD], BF16, tag="cos", name="cos")
            sin_sb = cpool.tile([128, TPAD], BF16, tag="sin", name="sin")
            idn_sb = cpool.tile([128, 128], BF16, tag="idn", name="idn")
            ones_sb = cpool.tile([1, TPAD], BF16, tag="ones", name="ones")

            nc.sync.dma_start(out=idn_sb[:], in_=idn_d[:])

            def _load_weights():
                for c in range(8):
                    nc.sync.dma_start(out=wqk_sb[c][:], in_=wqk_d[c * 128:(c + 1) * 128, :])
                    nc.sync.dma_start(out=wv_sb[c][:], in_=wv_d[c * 128:(c + 1) * 128, :])

            def _load_consts():
                for c in range(2):
                    nc.gpsimd.dma_start(out=wo_sb[c][:], in_=wo_d[c * 128:(c + 1) * 128, :])
                nc.gpsimd.dma_start(out=bqk_sb[:], in_=bqk_d[:])
                nc.gpsimd.dma_start(out=bv_sb[:], in_=bv_d[:])
                nc.gpsimd.dma_start(out=cos_sb[:], in_=cos_d[:])
                nc.gpsimd.dma_start(out=sin_sb[:], in_=sin_d[:])

            nc.vector.memset(ones_sb[:], 1.0)
            eps_sb = cpool.tile([128, 1], F32, tag="eps", name="eps")
            nc.vector.memset(eps_sb[:], 1e-5)
            ones64_sb = cpool.tile([128, 64], BF16, tag="ones64", name="ones64")
            nc.vector.memset(ones64_sb[:], 1.0)

            # ---------------- persistent activations ----------------
            xTg = [
                ppool.tile([128, 8 * tw], BF16, tag=f"xT{g}", name=f"xT{g}")
                for g, (to, tw) in enumerate(TG)
            ]
            xT3 = [
                xTg[g][:, :].rearrange("p (c t) -> p c t", c=8) for g in range(5)
            ]

            def xslice(c, jo, jw):
                g = _tg_of(jo)
                to, tw = TG[g]
                assert jo + jw <= to + tw
                return xT3[g][:, c, jo - to:jo - to + jw]

            # qkT tiles: 0,1 = q head-pairs (h01, h23); 2,3 = k head-pairs
            qkT_sb = [ppool.tile([128, TPAD], BF16, tag=f"qkT{f}", name=f"qkT{f}") for f in range(4)]
            vaug_sb = [ppool.tile([128, 260], BF16, tag=f"v{k}", name=f"v{k}") for k in range(NT)]

            # ---------------- phase A: LayerNorm + transpose (per tile) ----------
            def emit_ln_tile(i):
                xa = wpool.tile([128, D], F32, tag="xa", name="xa", bufs=3)
                if i < 16:
                    eng = nc.sync if i % 2 == 0 else nc.gpsimd
                    eng.dma_start(out=xa[:], in_=x_d[i * 128:(i + 1) * 128, :])
                else:
                    nc.vector.memset(xa[:], 0.0)
                    nc.sync.dma_start(out=xa[0:1, :], in_=x_d[2048:2049, :])
                if i == 0:
                    _load_weights()
                if i == 1:
                    _load_consts()
                stats = wpool.tile([128, 12], F32, tag="stats", name="stats", bufs=3)
                mv = wpool.tile([128, 2], F32, tag="mv", name="mv", bufs=4)
                nc.vector.bn_stats(stats[:, 0:6], xa[:, 0:512])
                nc.vector.bn_stats(stats[:, 6:12], xa[:, 512:1024])
                nc.vector.bn_aggr(mv[:], stats[:])
                std = wpool.tile([128, 1], F32, tag="std", name="std", bufs=4)
                rstd = wpool.tile([128, 1], F32, tag="rstd", name="rstd", bufs=4)
                nmurstd = wpool.tile([128, 1], F32, tag="murstd", name="nmurstd")
                nc.scalar.activation(std[:], mv[:, 1:2], AF.Sqrt, bias=eps_sb[:])
                nc.vector.reciprocal(rstd[:], std[:])
                nc.vector.scalar_tensor_tensor(
                    nmurstd[:], mv[:, 0:1], -1.0, rstd[:], OP.mult, OP.mult
                )
                # xn = rstd*x - mu*rstd on the scalar engine (idle in this phase)
                xn = wpool.tile([128, D], BF16, tag="xn", name="xn", bufs=4)
                nc.scalar.activation(
                    xn[:], xa[:], AF.Identity, bias=nmurstd[:], scale=rstd[:]
                )
                g = _tg_of(i * 128)
                to, tw = TG[g]
                for s in range(2):
                    pst = pspool.tile([128, 512], BF16, tag="dots", name="pst", bufs=2)
                    for c in range(4):
                        nc.tensor.transpose(
                            pst[:, c * 128:(c + 1) * 128],
                            xn[:, (4 * s + c) * 128:(4 * s + c + 1) * 128],
                            idn_sb[:],
                        )
                    nc.vector.tensor_copy(
                        xT3[g][:, 4 * s:4 * s + 4, i * 128 - to:(i + 1) * 128 - to],
                        pst[:, :].rearrange("p (c t) -> p c t", c=4),
                    )

            # ---------------- V projection for one k-tile ----------------
            def emit_v_tile(k):
                psv = pspool.tile([128, 256], F32, tag="dots", name="psv", bufs=2)
                for c in range(8):
                    nc.tensor.matmul(
                        psv[:],
                        xslice(c, k * 128, 128),
                        wv_sb[c][:],
                        start=(c == 0),
                        stop=False,
                    )
                nc.tensor.matmul(
                    psv[:],
                    ones_sb[:, k * 128:(k + 1) * 128],
                    bv_sb[:],
                    start=False,
                    stop=True,
                )
                va = vaug_sb[k]
                va3 = va[:, :].rearrange("p (a b) -> p a b", a=4)
                if k < 16:
                    nc.scalar.copy(
                        va3[:, :, 0:64], psv[:, :].rearrange("p (a b) -> p a b", a=4)
                    )
                    nc.vector.memset(va3[:, :, 64:65], 1.0)
                else:
                    # only token 2048 is real; zero rows kill padded keys
                    nc.vector.memset(va[:], 0.0)
                    va3r = va[0:1, :].rearrange("p (a b) -> p a b", a=4)
                    nc.scalar.copy(
                        va3r[:, :, 0:64],
                        psv[0:1, :].rearrange("p (a b) -> p a b", a=4),
                    )
                    nc.vector.memset(va3r[:, :, 64:65], 1.0)

            # ------------- Q/K projection + RoPE for one J5 chunk -------------
            def emit_qk_chunk(f, jo, jw, on_act):
                psq = pspool.tile([128, 512], F32, tag="dots", name="psq", bufs=2)
                for c in range(8):
                    nc.tensor.matmul(
                        psq[:, 0:jw],
                        wqk_sb[c][:, f * 128:(f + 1) * 128],
                        xslice(c, jo, jw),
                        start=(c == 0),
                        stop=False,
                    )
                nc.tensor.matmul(
                    psq[:, 0:jw],
                    bqk_sb[:, f * 128:(f + 1) * 128],
                    ones_sb[:, jo:jo + jw],
                    start=False,
                    stop=True,
                )
                qf = wpool.tile([128, 512], BF16, tag="qf", name="qf", bufs=3)
                if on_act:
                    nc.scalar.copy(qf[:, 0:jw], psq[:, 0:jw])
                else:
                    nc.vector.tensor_copy(qf[:, 0:jw], psq[:, 0:jw])
                # rotate-half partner via partition-shifting DMAs (spread queues)
                qs = wpool.tile([128, 512], BF16, tag="qs", name="qs", bufs=3)
                e0, e1 = (nc.sync, nc.gpsimd) if on_act else (nc.sync, nc.gpsimd)
                e0.dma_start(out=qs[0:32, 0:jw], in_=qf[32:64, 0:jw])
                e1.dma_start(out=qs[32:64, 0:jw], in_=qf[0:32, 0:jw])
                e0.dma_start(out=qs[64:96, 0:jw], in_=qf[96:128, 0:jw])
                e1.dma_start(out=qs[96:128, 0:jw], in_=qf[64:96, 0:jw])
                t1 = wpool.tile([128, 512], BF16, tag="t1", name="t1", bufs=2)
                t2 = wpool.tile([128, 512], BF16, tag="t2", name="t2", bufs=2)
                nc.vector.tensor_mul(
                    t1[:, 0:jw], qf[:, 0:jw], cos_sb[:, jo:jo + jw]
                )
                nc.vector.tensor_mul(
                    t2[:, 0:jw], qs[:, 0:jw], sin_sb[:, jo:jo + jw]
                )
                nc.vector.tensor_add(
                    qkT_sb[f][:, jo:jo + jw], t1[:, 0:jw], t2[:, 0:jw]
                )

            # ---------------- fused phase A+B pipeline ----------------
            for g in range(5):
                to, tw = TG[g]
                for i in range(to // 128, (to + tw) // 128):
                    emit_ln_tile(i)
                for k in range(to // 128, (to + tw) // 128):
                    emit_v_tile(k)
                emit_qk_chunk(0, to, tw, on_act=True)
                emit_qk_chunk(2, to, tw, on_act=True)

            # ---------------- attention ----------------
            def att_pair(jo, jw, p, side, po_every=2, po_rem=1):
                """Heads 2p, 2p+1: row-group-packed dots + exp + attn@v.
                Returns (oev per head, dnp merged-denominator tile)."""
                qt, kt = qkT_sb[p], qkT_sb[2 + p]
                pso = [
                    [
                        pspool.tile([65, 512], F32, tag="pso", name=f"pso{hh}{si}", bufs=4)
                        for si in range(len(_subs(jw)))
                    ]
                    for hh in range(2)
                ]
                dnp = wpool.tile([64, 1024], BF16, tag="dnp", name="dnp", bufs=2)

                def emit_attnv(k, exs):
                    for hh in range(2):
                        h = 2 * p + hh
                        for si, (so, sw) in enumerate(_subs(jw)):
                            nc.tensor.matmul(
                                pso[hh][si][:, 0:sw],
                                vaug_sb[k][:, h * 65:h * 65 + 65],
                                exs[hh][:, so:so + sw],
                                start=(k == 0),
                                stop=(k == NT - 1),
                                skip_group_check=True,
                            )

                # attn@v runs one k-tile behind dots/exp so the in-order PE
                # queue never stalls waiting on the current k-tile's exp
                prev = None
                for k in range(NT):
                    psds = [
                        pspool.tile([128, 1024], F32, tag="dots", name=f"psd{hh}", bufs=2)
                        for hh in range(2)
                    ]
                    # interleave the two heads' subs so their disjoint row
                    # groups overlap on the PE array
                    for so, sw in _subs(jw):
                        for hh in range(2):
                            po = hh * 64
                            nc.tensor.matmul(
                                psds[hh][:, so:so + sw],
                                kt[po:po + 64, k * 128:(k + 1) * 128],
                                qt[po:po + 64, jo + so:jo + so + sw],
                            )
                    exs = []
                    for hh in range(2):
                        ex = wpool.tile([128, 1024], BF16, tag="ex", name="ex", bufs=8)
                        nc.scalar.activation(
                            ex[:, 0:jw], psds[hh][:, 0:jw], AF.Exp, scale=SCALE
                        )
                        exs.append(ex)
                    if prev is not None:
                        emit_attnv(prev[0], prev[1])
                    if side and k % po_every == po_rem:
                        side.pop(0)()
                    prev = (k, exs)
                emit_attnv(prev[0], prev[1])
                oev_l = []
                for hh in range(2):
                    h = 2 * p + hh
                    oev = wpool.tile([65, 1024], BF16, tag="oev", name="oev", bufs=8)
                    for si, (so, sw) in enumerate(_subs(jw)):
                        nc.vector.tensor_copy(oev[:, so:so + sw], pso[hh][si][:, 0:sw])
                    oev_l.append(oev)
                    nc.gpsimd.dma_start(
                        out=dnp[32 * hh:32 * hh + 1, 0:jw],
                        in_=oev[64:65, 0:jw],
                    )
                return oev_l, dnp

            def att_head1(jo, h, pso_l, dnp_l):
                """Single-column (jw=1) attention for head h."""
                qt = qkT_sb[h // 2]
                ktile = qkT_sb[2 + h // 2]
                po = (h % 2) * 64
                pso = pspool.tile([65, 512], F32, tag="pso", name="pso1", bufs=4)
                psd = pspool.tile([128, NT], F32, tag="dots", name="psdt", bufs=2)
                for k in range(NT):
                    nc.tensor.matmul(
                        psd[:, k:k + 1],
                        ktile[po:po + 64, k * 128:(k + 1) * 128],
                        qt[po:po + 64, jo:jo + 1],
                    )
                ex = wpool.tile([128, 1024], BF16, tag="ex", name="ex", bufs=8)
                nc.scalar.activation(ex[:, 0:NT], psd[:], AF.Exp, scale=SCALE)
                for k in range(NT):
                    nc.tensor.matmul(
                        pso[:, 0:1],
                        vaug_sb[k][:, h * 65:h * 65 + 65],
                        ex[:, k:k + 1],
                        start=(k == 0),
                        stop=(k == NT - 1),
                        skip_group_check=True,
                    )
                oev = wpool.tile([65, 64], BF16, tag="oevt", name="oevt", bufs=4)
                nc.vector.tensor_copy(oev[:, 0:1], pso[:, 0:1])
                pso_l.append(oev)
                dnp = dnp_l[h // 2]
                nc.gpsimd.dma_start(
                    out=dnp[32 * (h % 2):32 * (h % 2) + 1, 0:1],
                    in_=oev[64:65, 0:1],
                )

            def epi_recip(jw, dnp):
                # one full-width bf16->f32 cast, approx reciprocal, cast back
                dnb = wpool.tile([64, 1024], BF16, tag="dnb", name="dnb", bufs=6)
                dnf = wpool.tile([64, 1024], F32, tag="dnf", name="dnf", bufs=1)
                dnr = wpool.tile([64, 1024], F32, tag="dnr", name="dnr", bufs=1)
                nc.vector.tensor_copy(dnf[:, 0:jw], dnp[:, 0:jw])
                nc.vector.reciprocal_approx_fast(out=dnr[:, 0:jw], in_=dnf[:, 0:jw])
                nc.vector.tensor_copy(dnb[:, 0:jw], dnr[:, 0:jw])
                return dnb

            def norm_chunk(h, so, sw, oev, dnb, an):
                hr = 32 * (h % 2)
                psb = pspool.tile([64, 512], F32, tag="dots", name="psb", bufs=2)
                nc.tensor.matmul(
                    psb[:, 0:sw],
                    ones64_sb[hr:hr + 1, :],
                    dnb[hr:hr + 1, so:so + sw],
                )
                if h % 2 == 0:
                    nc.vector.tensor_mul(
                        an[h // 2][0:64, so:so + sw],
                        oev[0:64, so:so + sw],
                        psb[:, 0:sw],
                    )
                else:
                    nt = wpool.tile([64, 512], BF16, tag="nt", name="nt", bufs=4)
                    nc.vector.tensor_mul(
                        nt[:, 0:sw], oev[0:64, so:so + sw], psb[:, 0:sw]
                    )
                    nc.gpsimd.dma_start(
                        out=an[h // 2][64:128, so:so + sw], in_=nt[:, 0:sw]
                    )

            def epi_norm(jw, p, oev_l, dnb, an):
                for hh in range(2):
                    for so, sw in _subs(jw):
                        norm_chunk(2 * p + hh, so, sw, oev_l[hh], dnb, an)

            def emit_op_chunk(jo, jw, an, of, so, sw, final=False):
                swr = min(sw, max(0, N - (jo + so)))
                pp = pspool.tile([128, 512], F32, tag="dots", name="pp", bufs=2)
                nc.tensor.matmul(
                    pp[:, 0:sw],
                    wo_sb[0][:, of * 128:(of + 1) * 128],
                    an[0][:, so:so + sw],
                    start=True,
                    stop=False,
                )
                nc.tensor.matmul(
                    pp[:, 0:sw],
                    wo_sb[1][:, of * 128:(of + 1) * 128],
                    an[1][:, so:so + sw],
                    start=False,
                    stop=True,
                )
                oo = wpool.tile([128, 512], BF16, tag="oo", name="oo", bufs=3)
                if final and of % 2 == 1:
                    nc.scalar.copy(oo[:, 0:swr], pp[:, 0:swr])
                else:
                    nc.vector.tensor_copy(oo[:, 0:swr], pp[:, 0:swr])
                nc.sync.dma_start(
                    out=out_d[of * 128:(of + 1) * 128, jo + so:jo + so + swr],
                    in_=oo[:, 0:swr],
                )

            # side work: Q/K projection + RoPE for heads 2-3, in half-chunks so
            # each injection fits the PE slack of one ACT-bound k-tile slot.
            # Order: K23 (pair23's dots need all of kT23), then Q23 g4 (the
            # cls-token block needs it), then Q23 g0-g3.
            side = []
            for f, g in ([(3, g) for g in range(5)] + [(1, 4)] +
                         [(1, g) for g in range(4)]):
                side.append(
                    lambda f=f, g=g: emit_qk_chunk(f, TG[g][0], TG[g][1], on_act=False)
                )

            def mk_an(nm):
                return [
                    wpool.tile([128, 1024], BF16, tag=f"an{nm}{ch}", name=f"an{nm}{ch}", bufs=1)
                    for ch in range(2)
                ]

            an_b0, an_b1, an_1 = mk_an("b0"), mk_an("b1"), mk_an("t")

            # ---- block b0 (q cols 0-1023): psq23 chunks side-injected ----
            oev01_b0, dnp0_b0 = att_pair(0, 1024, 0, side)
            oev23_b0, dnp1_b0 = att_pair(0, 1024, 1, side)
            # denominator reciprocals early (DVE-only, no PE queue blockage)
            dnb0_b0 = epi_recip(1024, dnp0_b0)
            dnb1_b0 = epi_recip(1024, dnp1_b0)

            # ---- jw=1 block (cls-token q column 2048) at the b0/b1 boundary ----
            dnp_l = [
                wpool.tile([64, 1024], BF16, tag="dnp", name=f"dnp1{p}", bufs=2)
                for p in range(2)
            ]
            oev1 = []
            for h in range(4):
                att_head1(2048, h, oev1, dnp_l)
            dnb_1 = [epi_recip(1, dnp_l[p]) for p in range(2)]

            def _norm_jw1():
                # deferred into b1-p01's side pops so the PE queue is not
                # blocked at the block boundary waiting on the DVE recip chain
                for p in range(2):
                    for hh in range(2):
                        norm_chunk(2 * p + hh, 0, 1, oev1[2 * p + hh], dnb_1[p], an_1)

            side += [_norm_jw1] + [
                (lambda of=of: emit_op_chunk(2048, 1, an_1, of, 0, 1))
                for of in range(8)
            ]

            # ---- block b1 (q cols 1024-2047): leftover side + jw1 outproj in
            #      p01, b0's epilogue between the pairs, b0's outproj in p23 ----
            oev01_b1, dnp0_b1 = att_pair(1024, 1024, 0, side, po_every=1, po_rem=0)
            epi_norm(1024, 0, oev01_b0, dnb0_b0, an_b0)
            epi_norm(1024, 1, oev23_b0, dnb1_b0, an_b0)
            side += [
                (lambda of=of, so=so, sw=sw:
                 emit_op_chunk(0, 1024, an_b0, of, so, sw))
                for of in range(8)
                for so, sw in _subs(1024)
            ]
            oev23_b1, dnp1_b1 = att_pair(1024, 1024, 1, side, po_every=1, po_rem=0)
            dnb0_b1 = epi_recip(1024, dnp0_b1)
            dnb1_b1 = epi_recip(1024, dnp1_b1)
            while side:
                side.pop(0)()
            # tail: normalize and out-project by column half so the second
            # half's norm (DVE) overlaps the first half's outproj (PE)
            for so, sw in _subs(1024):
                for hh in range(2):
                    norm_chunk(0 + hh, so, sw, oev01_b1[hh], dnb0_b1, an_b1)
                    norm_chunk(2 + hh, so, sw, oev23_b1[hh], dnb1_b1, an_b1)
                for of in range(8):
                    emit_op_chunk(1024, 1024, an_b1, of, so, sw,
                                  final=(of % 2 == 1))

    nc.compile()
    return nc


_NC = None


def _get_nc():
    global _NC
    if _NC is None:
        _NC = _build()
    return _NC


def _make_inputs(x, ln_gamma, ln_beta, w_qkv, w_out):
    w_eff = (w_qkv * ln_gamma[:, None].astype(np.float32)).astype(np.float32)
    b_all = (ln_beta.astype(np.float32) @ w_qkv.astype(np.float32)).astype(np.float32)

    inv = 1.0 / (10000.0 ** (np.arange(0, 64, 2, dtype=np.float64) / 64.0))
    fr = np.arange(2048, dtype=np.float64)[:, None] * inv[None, :]
    cos64 = np.concatenate([np.cos(fr), np.cos(fr)], axis=1).T  # [64, 2048]
    sinf64 = np.concatenate([-np.sin(fr), np.sin(fr)], axis=1).T
    cos2 = np.ones((128, TPAD), np.float32)
    sinf2 = np.zeros((128, TPAD), np.float32)
    cos2[:, 1:2049] = np.tile(cos64, (2, 1)).astype(np.float32)
    sinf2[:, 1:2049] = np.tile(sinf64, (2, 1)).astype(np.float32)
    ident = np.eye(128, dtype=BF)

    in_maps = []
    for c in range(8):
        b, g = c // 4, c % 4
        cols = slice(256 * g, 256 * g + 256)
        wqk = np.concatenate(
            [w_eff[:, 0:1024][:, cols], w_eff[:, 1024:2048][:, cols]], axis=1
        ).astype(BF)
        wv = w_eff[:, 2048:3072][:, cols].astype(BF)
        wo = w_out[cols, :].astype(BF)
        bqk = np.concatenate([b_all[0:1024][cols], b_all[1024:2048][cols]])[None, :].astype(BF)
        bv = b_all[2048:3072][cols][None, :].astype(BF)
        in_maps.append(
            {
                "x": np.ascontiguousarray(x[b]).astype(np.float32),
                "wqk": np.ascontiguousarray(wqk),
                "wv": np.ascontiguousarray(wv),
                "wo": np.ascontiguousarray(wo),
                "bqk": bqk,
                "bv": bv,
                "cos2": cos2.astype(BF),
                "sinf2": sinf2.astype(BF),
                "ident": ident,
            }
        )
    return in_maps


def kernel(x, ln_gamma, ln_beta, w_qkv, w_out, _trace=False, _trace_kwargs=None):
    nc = _get_nc()
    in_maps = _make_inputs(x, ln_gamma, ln_beta, w_qkv, w_out)
    res = run_bass_kernel_spmd(
        nc, in_maps, core_ids=list(range(8)), trace=_trace,
        **(_trace_kwargs or {}),
    )
    out = np.zeros((B, N, D), np.float32)
    for c in range(8):
        out[c // 4] += np.asarray(res.results[c]["out"], np.float32).T
    if _trace:
        return out, res
    return out


# revision 32
# speedup vs baseline: 1.2278x; 1.0806x over previous
"""Trainium2 Bass kernel for a ViT attention block (LN -> QKV -> RoPE -> attn -> out-proj).

Sharding: 8 cores = 2 batches x 4 head-groups (4 heads each). Each core computes
a partial out-projection (its 4 heads) for one batch, transposed as [D, N] bf16.
Host sums the 4 partials per batch and transposes back. LayerNorm gamma/beta are
folded into the QKV weights/bias on the host.

Deep-pipelined schedule (v2):
  - Per token group g (4+4+4+4+1 of the 17 token tiles): LN tiles (stats on DVE,
    normalize on ACT with per-partition scale/bias), PE-transpose into xT, then
    immediately V-projection for the group's k-tiles and Q/K projection+RoPE for
    heads 0-1 (f=0,2). So attention for head pair (0,1) starts as soon as group 4
    lands, ~3x earlier than a phase-sequential schedule.
  - Attention per jj-block per head pair: the two heads' K=64 dots matmuls sit in
    disjoint PE row groups (partitions 0:64 / 64:128, tile_position auto-derived)
    and are emitted back-to-back so they run concurrently. exp on ACT is the
    critical engine; attn@v (M=65 with appended ones column giving the softmax
    denominator) follows per head.
  - Q/K projection+RoPE for heads 2-3 (f=1,3) and block-0's out-projection are
    side-injected between k-tiles of the ACT-bound attention loops (PE/DVE slack).
  - Denominators: merged per pair into one tile, bf16->f32 cast, single
    reciprocal_approx_fast, cast back; normalize via ones outer-product broadcast
    (PE) + DVE multiply; out-proj accumulates the 256 head dims; out as [D,N] bf16.
PSUM: "dots" ring 2x[128,1024]f32 (4 banks; also transposes, side psq, pp, psb),
"pso" ring 4x[65,512]f32 (4 banks) for the pair's attn@v accumulators.
All matmuls bf16 with f32 PSUM accumulation.
"""

import sys

sys.path.insert(0, "/opt/trn_rl_repo")

import numpy as np
import ml_dtypes

import concourse.bacc as bacc
import concourse.mybir as mybir
import concourse.tile as tile
from concourse.bass_utils import run_bass_kernel_spmd

F32 = mybir.dt.float32
BF16 = mybir.dt.bfloat16
AF = mybir.ActivationFunctionType
OP = mybir.AluOpType
BF = ml_dtypes.bfloat16

B, N, D = 2, 2049, 1024
DH = 64
HPC = 4  # heads per core
NT = 17  # 128-token tiles (padded to 2176)
TPAD = NT * 128
SCALE = DH ** -0.5
# q-column blocks [offset, width]; the tail block is the single real token 2048
JJ = [(0, 1024), (1024, 1024), (2048, 1)]
J5 = [(0, 512), (512, 512), (1024, 512), (1536, 512), (2048, 128)]
# token groups backing the 5 xT tiles (4+4+4+4+1 of the 17 token tiles)
TG = [(0, 512), (512, 512), (1024, 512), (1536, 512), (2048, 128)]


def _subs(jw):
    return [(s, min(512, jw - s)) for s in range(0, jw, 512)]


def _tg_of(col):
    return min(col // 512, 4)


def _build():
    nc = bacc.Bacc("TRN2", target_bir_lowering=False, debug=False, num_devices=8)

    x_d = nc.declare_dram_parameter("x", [N, D], F32, False)
    wqk_d = nc.declare_dram_parameter("wqk", [D, 512], BF16, False)
    wv_d = nc.declare_dram_parameter("wv", [D, 256], BF16, False)
    wo_d = nc.declare_dram_parameter("wo", [256, D], BF16, False)
    bqk_d = nc.declare_dram_parameter("bqk", [1, 512], BF16, False)
    bv_d = nc.declare_dram_parameter("bv", [1, 256], BF16, False)
    cos_d = nc.declare_dram_parameter("cos2", [128, TPAD], BF16, False)
    sin_d = nc.declare_dram_parameter("sinf2", [128, TPAD], BF16, False)
    idn_d = nc.declare_dram_parameter("ident", [128, 128], BF16, False)
    out_d = nc.declare_dram_parameter("out", [D, N], BF16, True)

    with tile.TileContext(nc) as tc:
        with (
            tc.tile_pool(name="const", bufs=1) as cpool,
            tc.tile_pool(name="persist", bufs=1) as ppool,
            tc.tile_pool(name="work", bufs=2) as wpool,
            tc.tile_pool(name="psum", bufs=2, space="PSUM") as pspool,
        ):
            # ---------------- constants ----------------
            wqk_sb = [cpool.tile([128, 512], BF16, tag=f"wqk{c}", name=f"wqk{c}") for c in range(8)]
            wv_sb = [cpool.tile([128, 256], BF16, tag=f"wv{c}", name=f"wv{c}") for c in range(8)]
            wo_sb = [cpool.tile([128, 1024], BF16, tag=f"wo{c}", name=f"wo{c}") for c in range(2)]
            bqk_sb = cpool.tile([1, 512], BF16, tag="bqk", name="bqk")
            bv_sb = cpool.tile([1, 256], BF16, tag="bv", name="bv")
            cos_sb = cpool.tile([128, TPAD], BF16, tag="cos", name="cos")
            sin_sb = cpool.tile([128, TPAD], BF16, tag="sin", name="sin")
            idn_sb = cpool.tile([128, 128], BF16, tag="idn", name="idn")
            ones_sb = cpool.tile([1, TPAD], BF16, tag="ones", name="ones")

            nc.sync.dma_start(out=idn_sb[:], in_=idn_d[:])

            def _load_weights():
                for c in range(8):
                    nc.sync.dma_start(out=wqk_sb[c][:], in_=wqk_d[c * 128:(c + 1) * 128, :])
                    nc.sync.dma_start(out=wv_sb[c][:], in_=wv_d[c * 128:(c + 1) * 128, :])

            def _load_consts():
                for c in range(2):
                    nc.gpsimd.dma_start(out=wo_sb[c][:], in_=wo_d[c * 128:(c + 1) * 128, :])
                nc.gpsimd.dma_start(out=bqk_sb[:], in_=bqk_d[:])
                nc.gpsimd.dma_start(out=bv_sb[:], in_=bv_d[:])
                nc.gpsimd.dma_start(out=cos_sb[:], in_=cos_d[:])
                nc.gpsimd.dma_start(out=sin_sb[:], in_=sin_d[:])

            nc.vector.memset(ones_sb[:], 1.0)
            eps_sb = cpool.tile([128, 1], F32, tag="eps", name="eps")
            nc.vector.memset(eps_sb[:], 1e-5)
            ones64_sb = cpool.tile([128, 64], BF16, tag="ones64", name="ones64")
            nc.vector.memset(ones64_sb[:], 1.0)

            # ---------------- persistent activations ----------------
            xTg = [
                ppool.tile([128, 8 * tw], BF16, tag=f"xT{g}", name=f"xT{g}")
                for g, (to, tw) in enumerate(TG)
            ]
            xT3 = [
                xTg[g][:, :].rearrange("p (c t) -> p c t", c=8) for g in range(5)
            ]

            def xslice(c, jo, jw):
                g = _tg_of(jo)
                to, tw = TG[g]
                assert jo + jw <= to + tw
                return xT3[g][:, c, jo - to:jo - to + jw]

            # qkT tiles: 0,1 = q head-pairs (h01, h23); 2,3 = k head-pairs
            qkT_sb = [ppool.tile([128, TPAD], BF16, tag=f"qkT{f}", name=f"qkT{f}") for f in range(4)]
            vaug_sb = [ppool.tile([128, 260], BF16, tag=f"v{k}", name=f"v{k}") for k in range(NT)]

            # ---------------- phase A: LayerNorm + transpose (per tile) ----------
            def emit_ln_tile(i):
                xa = wpool.tile([128, D], F32, tag="xa", name="xa", bufs=3)
                if i < 16:
                    eng = nc.sync if i % 2 == 0 else nc.gpsimd
                    eng.dma_start(out=xa[:], in_=x_d[i * 128:(i + 1) * 128, :])
                else:
                    nc.vector.memset(xa[:], 0.0)
                    nc.sync.dma_start(out=xa[0:1, :], in_=x_d[2048:2049, :])
                if i == 0:
                    _load_weights()
                if i == 1:
                    _load_consts()
                stats = wpool.tile([128, 12], F32, tag="stats", name="stats", bufs=3)
                mv = wpool.tile([128, 2], F32, tag="mv", name="mv", bufs=4)
                nc.vector.bn_stats(stats[:, 0:6], xa[:, 0:512])
                nc.vector.bn_stats(stats[:, 6:12], xa[:, 512:1024])
                nc.vector.bn_aggr(mv[:], stats[:])
                std = wpool.tile([128, 1], F32, tag="std", name="std", bufs=4)
                rstd = wpool.tile([128, 1], F32, tag="rstd", name="rstd", bufs=4)
                nmurstd = wpool.tile([128, 1], F32, tag="murstd", name="nmurstd")
                nc.scalar.activation(std[:], mv[:, 1:2], AF.Sqrt, bias=eps_sb[:])
                nc.vector.reciprocal(rstd[:], std[:])
                nc.vector.scalar_tensor_tensor(
                    nmurstd[:], mv[:, 0:1], -1.0, rstd[:], OP.mult, OP.mult
                )
                # xn = rstd*x - mu*rstd on the scalar engine (idle in this phase)
                xn = wpool.tile([128, D], BF16, tag="xn", name="xn", bufs=4)
                nc.scalar.activation(
                    xn[:], xa[:], AF.Identity, bias=nmurstd[:], scale=rstd[:]
                )
                g = _tg_of(i * 128)
                to, tw = TG[g]
                for s in range(2):
                    pst = pspool.tile([128, 512], BF16, tag="dots", name="pst", bufs=3)
                    for c in range(4):
                        nc.tensor.transpose(
                            pst[:, c * 128:(c + 1) * 128],
                            xn[:, (4 * s + c) * 128:(4 * s + c + 1) * 128],
                            idn_sb[:],
                        )
                    nc.vector.tensor_copy(
                        xT3[g][:, 4 * s:4 * s + 4, i * 128 - to:(i + 1) * 128 - to],
                        pst[:, :].rearrange("p (c t) -> p c t", c=4),
                    )

            # ---------------- V projection for one k-tile ----------------
            def emit_v_tile(k):
                psv = pspool.tile([128, 256], F32, tag="po", name="psv", bufs=2)
                for c in range(8):
                    nc.tensor.matmul(
                        psv[:],
                        xslice(c, k * 128, 128),
                        wv_sb[c][:],
                        start=(c == 0),
                        stop=False,
                    )
                nc.tensor.matmul(
                    psv[:],
                    ones_sb[:, k * 128:(k + 1) * 128],
                    bv_sb[:],
                    start=False,
                    stop=True,
                )
                va = vaug_sb[k]
                va3 = va[:, :].rearrange("p (a b) -> p a b", a=4)
                if k < 16:
                    nc.scalar.copy(
                        va3[:, :, 0:64], psv[:, :].rearrange("p (a b) -> p a b", a=4)
                    )
                    nc.vector.memset(va3[:, :, 64:65], 1.0)
                else:
                    # only token 2048 is real; zero rows kill padded keys
                    nc.vector.memset(va[:], 0.0)
                    va3r = va[0:1, :].rearrange("p (a b) -> p a b", a=4)
                    nc.scalar.copy(
                        va3r[:, :, 0:64],
                        psv[0:1, :].rearrange("p (a b) -> p a b", a=4),
                    )
                    nc.vector.memset(va3r[:, :, 64:65], 1.0)

            # ------------- Q/K projection + RoPE for one J5 chunk -------------
            def emit_qk_chunk(f, jo, jw, on_act):
                psq = pspool.tile([128, 512], F32, tag="po", name="psq", bufs=2)
                for c in range(8):
                    nc.tensor.matmul(
                        psq[:, 0:jw],
                        wqk_sb[c][:, f * 128:(f + 1) * 128],
                        xslice(c, jo, jw),
                        start=(c == 0),
                        stop=False,
                    )
                nc.tensor.matmul(
                    psq[:, 0:jw],
                    bqk_sb[:, f * 128:(f + 1) * 128],
                    ones_sb[:, jo:jo + jw],
                    start=False,
                    stop=True,
                )
                qf = wpool.tile([128, 512], BF16, tag="qf", name="qf", bufs=3)
                if on_act:
                    nc.scalar.copy(qf[:, 0:jw], psq[:, 0:jw])
                else:
                    nc.vector.tensor_copy(qf[:, 0:jw], psq[:, 0:jw])
                # rotate-half partner via partition-shifting DMAs (spread queues)
                qs = wpool.tile([128, 512], BF16, tag="qs", name="qs", bufs=3)
                e0, e1 = (nc.sync, nc.gpsimd) if on_act else (nc.sync, nc.gpsimd)
                e0.dma_start(out=qs[0:32, 0:jw], in_=qf[32:64, 0:jw])
                e1.dma_start(out=qs[32:64, 0:jw], in_=qf[0:32, 0:jw])
                e0.dma_start(out=qs[64:96, 0:jw], in_=qf[96:128, 0:jw])
                e1.dma_start(out=qs[96:128, 0:jw], in_=qf[64:96, 0:jw])
                t1 = wpool.tile([128, 512], BF16, tag="t1", name="t1", bufs=2)
                t2 = wpool.tile([128, 512], BF16, tag="t2", name="t2", bufs=2)
                nc.vector.tensor_mul(
                    t1[:, 0:jw], qf[:, 0:jw], cos_sb[:, jo:jo + jw]
                )
                nc.vector.tensor_mul(
                    t2[:, 0:jw], qs[:, 0:jw], sin_sb[:, jo:jo + jw]
                )
                nc.vector.tensor_add(
                    qkT_sb[f][:, jo:jo + jw], t1[:, 0:jw], t2[:, 0:jw]
                )

            # ---------------- fused phase A+B pipeline ----------------
            for g in range(5):
                to, tw = TG[g]
                for i in range(to // 128, (to + tw) // 128):
                    emit_ln_tile(i)
                for k in range(to // 128, (to + tw) // 128):
                    emit_v_tile(k)
                emit_qk_chunk(0, to, tw, on_act=True)
                emit_qk_chunk(2, to, tw, on_act=True)

            # ---------------- attention (sequential heads) ----------------
            # Per head: dots (K=64, row-group auto-derived from partition
            # offset), exp on ACT (the critical engine), attn@v (M=65, ones
            # column = softmax denominator). Side work (heads-2/3 projection
            # chunks, then the previous block's out-projection) pops every
            # 4th k-tile into the PE/DVE slack of the ACT-bound loop.
            def att_head(jo, jw, h, oev_l, dnp, side):
                qt = qkT_sb[h // 2]
                ktile = qkT_sb[2 + h // 2]
                po = (h % 2) * 64
                pso = [
                    pspool.tile([65, 512], F32, tag="po", name=f"pso{si}", bufs=2)
                    for si in range(2 if jw > 1 else 1)
                ]
                if jw == 1:
                    psd = pspool.tile([128, NT], F32, tag="dots", name="psdt", bufs=3)
                    for k in range(NT):
                        nc.tensor.matmul(
                            psd[:, k:k + 1],
                            ktile[po:po + 64, k * 128:(k + 1) * 128],
                            qt[po:po + 64, jo:jo + 1],
                        )
                    ex = wpool.tile([128, 1024], BF16, tag="ex", name="ex", bufs=8)
                    nc.scalar.activation(ex[:, 0:NT], psd[:], AF.Exp, scale=SCALE)
                    for k in range(NT):
                        nc.tensor.matmul(
                            pso[0][:, 0:1],
                            vaug_sb[k][:, h * 65:h * 65 + 65],
                            ex[:, k:k + 1],
                            start=(k == 0),
                            stop=(k == NT - 1),
                            skip_group_check=True,
                        )
                        if side and k % 4 == 3:
                            side.pop(0)()
                else:
                    for k in range(NT):
                        psd = pspool.tile([128, 1024], F32, tag="dots", name="psd", bufs=3)
                        for so, sw in _subs(jw):
                            nc.tensor.matmul(
                                psd[:, so:so + sw],
                                ktile[po:po + 64, k * 128:(k + 1) * 128],
                                qt[po:po + 64, jo + so:jo + so + sw],
                            )
                        ex = wpool.tile([128, 1024], BF16, tag="ex", name="ex", bufs=8)
                        nc.scalar.activation(
                            ex[:, 0:jw], psd[:, 0:jw], AF.Exp, scale=SCALE
                        )
                        for si, (so, sw) in enumerate(_subs(jw)):
                            nc.tensor.matmul(
                                pso[si][:, 0:sw],
                                vaug_sb[k][:, h * 65:h * 65 + 65],
                                ex[:, so:so + sw],
                                start=(k == 0),
                                stop=(k == NT - 1),
                                skip_group_check=True,
                            )
                        if side and k % 4 == 3:
                            side.pop(0)()
                oev = (wpool.tile([65, 64], BF16, tag="oevt", name="oevt", bufs=4)
                       if jw == 1 else
                       wpool.tile([65, 1024], BF16, tag="oev", name="oev", bufs=4))
                for si, (so, sw) in enumerate(_subs(jw)):
                    nc.vector.tensor_copy(oev[:, so:so + sw], pso[si][:, 0:sw])
                oev_l.append(oev)
                nc.gpsimd.dma_start(
                    out=dnp[h // 2][32 * (h % 2):32 * (h % 2) + 1, 0:jw],
                    in_=oev[64:65, 0:jw],
                )

            def epi_recip(jw, dnp1):
                # one full-width bf16->f32 cast, approx reciprocal, cast back
                dnb = wpool.tile([64, 1024], BF16, tag="dnb", name="dnb", bufs=6)
                dnf = wpool.tile([64, 1024], F32, tag="dnf", name="dnf", bufs=1)
                dnr = wpool.tile([64, 1024], F32, tag="dnr", name="dnr", bufs=1)
                nc.vector.tensor_copy(dnf[:, 0:jw], dnp1[:, 0:jw])
                nc.vector.reciprocal_approx_fast(out=dnr[:, 0:jw], in_=dnf[:, 0:jw])
                nc.vector.tensor_copy(dnb[:, 0:jw], dnr[:, 0:jw])
                return dnb

            def norm_chunk(h, so, sw, oev, dnb, an):
                hr = 32 * (h % 2)
                psb = pspool.tile([64, 512], F32, tag="po", name="psb", bufs=2)
                nc.tensor.matmul(
                    psb[:, 0:sw],
                    ones64_sb[hr:hr + 1, :],
                    dnb[hr:hr + 1, so:so + sw],
                )
                if h % 2 == 0:
                    nc.vector.tensor_mul(
                        an[h // 2][0:64, so:so + sw],
                        oev[0:64, so:so + sw],
                        psb[:, 0:sw],
                    )
                else:
                    nt = wpool.tile([64, 512], BF16, tag="nt", name="nt", bufs=4)
                    nc.vector.tensor_mul(
                        nt[:, 0:sw], oev[0:64, so:so + sw], psb[:, 0:sw]
                    )
                    nc.gpsimd.dma_start(
                        out=an[h // 2][64:128, so:so + sw], in_=nt[:, 0:sw]
                    )

            def epi_pair(jw, p2, oev_l, dnp, an):
                dnb = epi_recip(jw, dnp[p2])
                for hh in range(2):
                    for so, sw in _subs(jw):
                        norm_chunk(2 * p2 + hh, so, sw, oev_l[2 * p2 + hh], dnb, an)

            def emit_op_chunk(jo, jw, an, of, so, sw, final=False):
                swr = min(sw, max(0, N - (jo + so)))
                pp = pspool.tile([128, 512], F32, tag="dots", name="pp", bufs=3)
                nc.tensor.matmul(
                    pp[:, 0:sw],
                    wo_sb[0][:, of * 128:(of + 1) * 128],
                    an[0][:, so:so + sw],
                    start=True,
                    stop=False,
                )
                nc.tensor.matmul(
                    pp[:, 0:sw],
                    wo_sb[1][:, of * 128:(of + 1) * 128],
                    an[1][:, so:so + sw],
                    start=False,
                    stop=True,
                )
                oo = wpool.tile([128, 512], BF16, tag="oo", name="oo", bufs=3)
                if final and of % 2 == 1:
                    nc.scalar.copy(oo[:, 0:swr], pp[:, 0:swr])
                else:
                    nc.vector.tensor_copy(oo[:, 0:swr], pp[:, 0:swr])
                eng = nc.sync if of % 2 == 0 else nc.gpsimd
                eng.dma_start(
                    out=out_d[of * 128:(of + 1) * 128, jo + so:jo + so + swr],
                    in_=oo[:, 0:swr],
                )

            # side work: heads-2/3 projection chunks (K23 first — every k-tile
            # of pair-23 dots needs kT23; then Q23 g4 for the cls block)
            side = []
            for f, g in ([(3, g) for g in range(5)] + [(1, 4)] +
                         [(1, g) for g in range(4)]):
                side.append(
                    lambda f=f, g=g: emit_qk_chunk(f, TG[g][0], TG[g][1], on_act=False)
                )

            for jo, jw in JJ:
                an = [
                    wpool.tile([128, 1024], BF16, tag=f"an{ch}", name=f"an{ch}", bufs=2)
                    for ch in range(2)
                ]
                dnp = [
                    wpool.tile([64, 1024], BF16, tag=f"dnp{p2}", name=f"dnp{p2}", bufs=2)
                    for p2 in range(2)
                ]
                oev_l = []
                att_head(jo, jw, 0, oev_l, dnp, side)
                att_head(jo, jw, 1, oev_l, dnp, side)
                att_head(jo, jw, 2, oev_l, dnp, side)
                epi_pair(jw, 0, oev_l, dnp, an)
                att_head(jo, jw, 3, oev_l, dnp, side)
                epi_pair(jw, 1, oev_l, dnp, an)
                while side:
                    side.pop(0)()
                side = [
                    (lambda fin=False, jo=jo, jw=jw, an=an, of=of, so=so, sw=sw:
                     emit_op_chunk(jo, jw, an, of, so, sw, fin))
                    for of in range(8)
                    for so, sw in _subs(jw)
                ]
            for item in side:
                item(True)

    nc.compile()
    return nc


_NC = None


def _get_nc():
    global _NC
    if _NC is None:
        _NC = _build()
    return _NC


def _make_inputs(x, ln_gamma, ln_beta, w_qkv, w_out):
    w_eff = (w_qkv * ln_gamma[:, None].astype(np.float32)).astype(np.float32)
    b_all = (ln_beta.astype(np.float32) @ w_qkv.astype(np.float32)).astype(np.float32)

    inv = 1.0 / (10000.0 ** (np.arange(0, 64, 2, dtype=np.float64) / 64.0))
    fr = np.arange(2048, dtype=np.float64)[:, None] * inv[None, :]
    cos64 = np.concatenate([np.cos(fr), np.cos(fr)], axis=1).T  # [64, 2048]
    sinf64 = np.concatenate([-np.sin(fr), np.sin(fr)], axis=1).T
    cos2 = np.ones((128, TPAD), np.float32)
    sinf2 = np.zeros((128, TPAD), np.float32)
    cos2[:, 1:2049] = np.tile(cos64, (2, 1)).astype(np.float32)
    sinf2[:, 1:2049] = np.tile(sinf64, (2, 1)).astype(np.float32)
    ident = np.eye(128, dtype=BF)

    in_maps = []
    for c in range(8):
        b, g = c // 4, c % 4
        cols = slice(256 * g, 256 * g + 256)
        wqk = np.concatenate(
            [w_eff[:, 0:1024][:, cols], w_eff[:, 1024:2048][:, cols]], axis=1
        ).astype(BF)
        wv = w_eff[:, 2048:3072][:, cols].astype(BF)
        wo = w_out[cols, :].astype(BF)
        bqk = np.concatenate([b_all[0:1024][cols], b_all[1024:2048][cols]])[None, :].astype(BF)
        bv = b_all[2048:3072][cols][None, :].astype(BF)
        in_maps.append(
            {
                "x": np.ascontiguousarray(x[b]).astype(np.float32),
                "wqk": np.ascontiguousarray(wqk),
                "wv": np.ascontiguousarray(wv),
                "wo": np.ascontiguousarray(wo),
                "bqk": bqk,
                "bv": bv,
                "cos2": cos2.astype(BF),
                "sinf2": sinf2.astype(BF),
                "ident": ident,
            }
        )
    return in_maps


def kernel(x, ln_gamma, ln_beta, w_qkv, w_out, _trace=False, _trace_kwargs=None):
    nc = _get_nc()
    in_maps = _make_inputs(x, ln_gamma, ln_beta, w_qkv, w_out)
    res = run_bass_kernel_spmd(
        nc, in_maps, core_ids=list(range(8)), trace=_trace,
        **(_trace_kwargs or {}),
    )
    out = np.zeros((B, N, D), np.float32)
    for c in range(8):
        out[c // 4] += np.asarray(res.results[c]["out"], np.float32).T
    if _trace:
        return out, res
    return out
